# revision 6
# baseline (speedup 1.0000x reference)
"""GateTypeExpertLayer kernel for 8 Trainium2 NeuronCores (SPMD).

v4: instruction-count-minimized design. Through this execution stack every
device instruction costs ~25-70us (measured: DVE ~26us, matmul ~69us,
DMA ~52us), so the kernel is built to minimize instruction count:

  - Host computes routing exactly (histogram -> router logits -> top-2 ->
    softmax gates) and sorts nodes by their unordered expert *pair* so each
    contiguous slot-run needs exactly 2 experts. Host also un-permutes the
    output. (The previous baseline already hosted the histogram + routing
    plan; this moves the rest of the routing bookkeeping there too.)
  - Device: per strip (<=512 slots, one expert pair): 4 W1 matmuls ->
    one batched Gelu -> one batched scale by per-slot gate weights
    (broadcast via a step-0-partition DMA load) -> 4 W2 matmuls that
    accumulate BOTH experts into one PSUM tile (the top-2 combine is free,
    done by PSUM accumulation on pre-scaled activations) -> one copy into
    a resident [128, S] bf16 accumulator.
  - One dma_start_transpose converts feat-major [128, S] to node-partition
    [128, S/128, 128]; LayerNorm runs as ~8 whole-tensor instructions; one
    contiguous DMA writes the output.

Per core: ~450 instructions total (vs ~2900 in the previous version).
"""

import numpy as np
import sys

sys.path.insert(0, "/opt/trn_rl_repo")

N_CORES = 8
N = 100000
H = 128
NUM_EXPERTS = 8
NUM_GATE_TYPES = 20
LN_EPS = 1e-5
NSH = N // N_CORES            # 12500 nodes per core
P = 128
STRIP = 512                   # max matmul free dim / PSUM bank
MAX_S = 18944                 # SBUF budget cap on padded slots per core

_PROGRAM_CACHE = {}


def _histogram(edge_index, edge_gate_type):
    dst = np.asarray(edge_index)[1].astype(np.int64)
    egt = np.asarray(edge_gate_type).astype(np.int64)
    return np.bincount(dst * NUM_GATE_TYPES + egt,
                       minlength=N * NUM_GATE_TYPES).reshape(
                           N, NUM_GATE_TYPES).astype(np.float32)


def _route(x, C, gate_type_embed, Wr, br):
    """Replicate the reference router in fp32 on host.

    Returns eA, eB (top-2 expert ids) and wA, wB (softmax gates)."""
    x = np.asarray(x, dtype=np.float32)
    G = np.asarray(gate_type_embed, dtype=np.float32)
    cnt = C.sum(axis=1, dtype=np.float32)
    gate = np.where(cnt[:, None] > 0,
                    (C @ G) / np.maximum(cnt, 1.0)[:, None],
                    0.0).astype(np.float32)
    logits = (x @ np.asarray(Wr, np.float32)
              + np.asarray(br, np.float32)[None, :] + gate)
    order = np.argsort(-logits, axis=1, kind="stable")
    eA = order[:, 0]
    eB = order[:, 1]
    lA = np.take_along_axis(logits, eA[:, None], 1)[:, 0].astype(np.float64)
    lB = np.take_along_axis(logits, eB[:, None], 1)[:, 0].astype(np.float64)
    wA = (1.0 / (1.0 + np.exp(lB - lA))).astype(np.float32)
    wB = (1.0 - wA).astype(np.float32)
    return eA, eB, wA, wB


def _plan(eA, eB):
    """Pair-sort plan shared by all cores (SPMD: one program).

    Nodes are assigned to cores round-robin *within each expert pair* (the
    host un-permutes afterwards, so any node->core assignment is legal).
    That balances pair counts exactly, so the shared per-pair capacity is
    ceil(total/8) with no alignment padding.

    Returns (strips, S, per_core): strips is a tuple of
    (offset, n, expert_a, expert_b) compile-time constants; per_core[i]
    holds (slot_node, valid, is_A_first) indexing arrays."""
    u = np.minimum(eA, eB)
    v = np.maximum(eA, eB)
    key = (u * NUM_EXPERTS + v).astype(np.int64)

    totals = np.bincount(key, minlength=NUM_EXPERTS * NUM_EXPERTS)
    cap = -(-totals // N_CORES)                      # ceil(total/8), exact
    active = np.where(cap > 0)[0]

    strips = []
    seg_base = {}
    off = 0
    for kk in active:
        seg_base[int(kk)] = off
        a, b = int(kk) // NUM_EXPERTS, int(kk) % NUM_EXPERTS
        rem = int(cap[kk])
        o = off
        while rem > 0:
            n = min(STRIP, rem)
            strips.append((o, n, a, b))
            o += n
            rem -= n
        off += int(cap[kk])
    S_raw = off
    S = -(-S_raw // P) * P                           # final 128-pad only

    per_core = [[np.zeros(S, np.int64), np.zeros(S, bool), np.zeros(S, bool)]
                for _ in range(N_CORES)]
    for kk in active:
        nodes = np.where(key == kk)[0]
        base = seg_base[int(kk)]
        a = int(kk) // NUM_EXPERTS
        for i in range(N_CORES):
            sel = nodes[i::N_CORES]
            c = len(sel)
            slot_node, valid, is_A_first = per_core[i]
            slot_node[base:base + c] = sel
            valid[base:base + c] = True
            is_A_first[base:base + c] = (eA[sel] == a)
    per_core = [tuple(pc) for pc in per_core]
    return tuple(strips), S, S_raw, per_core


def _build_v4(strips, S, S_raw=None, reps=1):
    if S_raw is None:
        S_raw = S
    import concourse.bacc as bacc
    import concourse.tile as tile
    import concourse.mybir as mybir
    import concourse.bass as bass

    f32 = mybir.dt.float32
    bf16 = mybir.dt.bfloat16
    AF = mybir.ActivationFunctionType
    OP = mybir.AluOpType
    G = S // P

    nc = bacc.Bacc("TRN2", target_bir_lowering=False, debug=False,
                   num_devices=N_CORES)

    xg = nc.dram_tensor("xg", [P, S], bf16, kind="ExternalInput").ap()
    wgd = nc.dram_tensor("wgd", [2, S], bf16, kind="ExternalInput").ap()
    w1s = nc.dram_tensor("w1s", [P, 2048], bf16, kind="ExternalInput").ap()
    w2s = nc.dram_tensor("w2s", [P, 2048], bf16, kind="ExternalInput").ap()
    outd = nc.dram_tensor("outd", [P, G, H], bf16, kind="ExternalOutput").ap()

    def pbc(sl, count):
        # DRAM partition-broadcast: read one row into all partitions
        ap = [list(d) for d in sl.ap]
        return bass.AP(tensor=sl.tensor, offset=sl.offset,
                       ap=[[0, count]] + ap[1:])

    def bc(sl, count):
        ap = [list(d) for d in sl.ap]
        return bass.AP(tensor=sl.tensor, offset=sl.offset,
                       ap=ap + [[0, count]])

    def wexp(sl):
        # [P, 2, n] -> [P, 2, 2, n] with a step-0 dim for the hidden halves
        ap = [list(d) for d in sl.ap]
        return bass.AP(tensor=sl.tensor, offset=sl.offset,
                       ap=[ap[0], ap[1], [0, 2], ap[2]])

    with tile.TileContext(nc) as tc:
        with tc.tile_pool(name="const", bufs=1) as constp:
            w1_sb = constp.tile([P, 2048], bf16)
            nc.sync.dma_start(out=w1_sb[:], in_=w1s[:])
            w2_sb = constp.tile([P, 2048], bf16)
            nc.sync.dma_start(out=w2_sb[:], in_=w2s[:])
            eps_sb = constp.tile([P, 1], f32)
            nc.vector.memset(eps_sb[:], LN_EPS)
            xg_sb = constp.tile([P, S], bf16)
            nc.sync.dma_start(out=xg_sb[:], in_=xg[:])
            # per-slot gate weights broadcast to all 128 partitions
            wball = constp.tile([P, 2, S], bf16)
            nc.sync.dma_start(out=wball[:, 0, :], in_=pbc(wgd[0:1, :], P))
            nc.sync.dma_start(out=wball[:, 1, :], in_=pbc(wgd[1:2, :], P))

            def _body():
                with tc.tile_pool(name="work", bufs=1) as wp, \
                     tc.tile_pool(name="hsp", bufs=2) as hsp, \
                     tc.tile_pool(name="hpsum", bufs=1, space="PSUM") as hps, \
                     tc.tile_pool(name="ypsum", bufs=2, space="PSUM") as yps:
                    yAll = wp.tile([P, S], bf16, tag="big")
                    if S > S_raw:
                        nc.vector.memset(yAll[:, S_raw:S], 0.0)
                    for (off, n, a, b) in strips:
                        hp = hps.tile([P, 2, 2, STRIP], f32, tag="hp")
                        for ei, e in enumerate((a, b)):
                            for m in range(2):
                                nc.tensor.matmul(
                                    out=hp[:, ei, m, 0:n],
                                    lhsT=w1_sb[:, e * 256 + m * P:
                                               e * 256 + (m + 1) * P],
                                    rhs=xg_sb[:, off:off + n],
                                    start=True, stop=True)
                        hs = hsp.tile([P, 2, 2, STRIP], bf16, tag="hs")
                        nc.scalar.activation(out=hs[:, :, :, 0:n],
                                             in_=hp[:, :, :, 0:n],
                                             func=AF.Gelu)
                        nc.vector.tensor_tensor(
                            out=hs[:, :, :, 0:n], in0=hs[:, :, :, 0:n],
                            in1=wexp(wball[:, :, off:off + n]), op=OP.mult)
                        yT = yps.tile([P, STRIP], f32, tag="yT")
                        k = 0
                        for ei, e in enumerate((a, b)):
                            for m in range(2):
                                nc.tensor.matmul(
                                    out=yT[:, 0:n],
                                    lhsT=w2_sb[:, (2 * e + m) * P:
                                               (2 * e + m + 1) * P],
                                    rhs=hs[:, ei, m, 0:n],
                                    start=(k == 0), stop=(k == 3))
                                k += 1
                        nc.vector.tensor_copy(out=yAll[:, off:off + n],
                                              in_=yT[:, 0:n])

                    # ---- LayerNorm over all nodes, then store ----
                    yn = wp.tile([P, G, H], bf16, tag="yn")
                    nc.sync.dma_start_transpose(yn[:], yAll[:])
                    mu = wp.tile([P, G], f32, tag="mu")
                    nc.vector.tensor_reduce(out=mu[:], in_=yn[:],
                                            axis=mybir.AxisListType.X,
                                            op=OP.add)
                    nc.vector.tensor_scalar_mul(mu[:], mu[:], 1.0 / H)
                    nc.vector.tensor_tensor(out=yn[:], in0=yn[:],
                                            in1=bc(mu[:], H), op=OP.subtract)
                    sq = wp.tile([P, G, H], bf16, tag="big")
                    nc.scalar.activation(out=sq[:], in_=yn[:], func=AF.Square)
                    vr = wp.tile([P, G], f32, tag="vr")
                    nc.vector.tensor_reduce(out=vr[:], in_=sq[:],
                                            axis=mybir.AxisListType.X,
                                            op=OP.add)
                    sd = wp.tile([P, G], f32, tag="sd")
                    nc.scalar.activation(out=sd[:], in_=vr[:], func=AF.Sqrt,
                                         bias=eps_sb[:], scale=1.0 / H)
                    nc.vector.reciprocal(sd[:], sd[:])
                    nc.vector.tensor_tensor(out=yn[:], in0=yn[:],
                                            in1=bc(sd[:], H), op=OP.mult)
                    nc.sync.dma_start(out=outd[:], in_=yn[:])

            for _rep in range(reps):
                _body()

    nc.compile()
    return nc


def _prep(x, eA, eB, wA, wB, W1, W2, strips, S, per_core):
    import ml_dtypes
    bf = ml_dtypes.bfloat16
    x = np.asarray(x, dtype=np.float32)
    W1 = np.asarray(W1, dtype=np.float32)
    W2 = np.asarray(W2, dtype=np.float32)

    w1s = W1.transpose(1, 0, 2).reshape(P, NUM_EXPERTS * 256).astype(bf)
    w2s = W2.reshape(NUM_EXPERTS, 2, P, H).transpose(2, 0, 1, 3).reshape(
        P, NUM_EXPERTS * 256).astype(bf)

    in_maps = []
    for i in range(N_CORES):
        slot_node, valid, is_A_first = per_core[i]
        xg = np.zeros((P, S), dtype=bf)
        xg[:, valid] = x[slot_node[valid]].T.astype(bf)
        # row 0: weight of pair-min expert; row 1: weight of pair-max expert
        wgd = np.zeros((2, S), dtype=np.float32)
        wa = wA[slot_node[valid]]
        wb = wB[slot_node[valid]]
        first = is_A_first[valid]
        wgd[0, valid] = np.where(first, wa, wb)
        wgd[1, valid] = np.where(first, wb, wa)
        in_maps.append({
            "xg": np.ascontiguousarray(xg),
            "wgd": np.ascontiguousarray(wgd.astype(bf)),
            "w1s": np.ascontiguousarray(w1s),
            "w2s": np.ascontiguousarray(w2s),
        })
    return in_maps


def _fallback_numpy(x, edge_gate_type, edge_index, gate_type_embed, Wr, br,
                    W1, b1, W2, b2, ln_gamma, ln_beta):
    # exact reference recomputation on host (only for unexpected inputs)
    import math
    x = np.asarray(x, dtype=np.float32)
    n = x.shape[0]
    C = _histogram(edge_index, edge_gate_type)
    G = np.asarray(gate_type_embed, dtype=np.float32)
    cnt = C.sum(axis=1, dtype=np.float32)
    gate = np.where(cnt[:, None] > 0,
                    (C @ G) / np.maximum(cnt, 1.0)[:, None], 0.0)
    rl = x @ np.asarray(Wr, np.float32) + np.asarray(br, np.float32) + gate
    order = np.argsort(-rl, axis=1, kind="stable")
    tki = order[:, :2]
    tkl = np.take_along_axis(rl, tki, 1)
    m = tkl.max(axis=1, keepdims=True)
    e = np.exp(tkl - m)
    tkg = e / e.sum(axis=1, keepdims=True)
    W1 = np.asarray(W1, np.float32)
    b1 = np.asarray(b1, np.float32)
    W2 = np.asarray(W2, np.float32)
    b2 = np.asarray(b2, np.float32)
    out = np.zeros((n, H), np.float32)
    from scipy.special import erf  # noqa: F401  (fallback only)
    for kk in range(2):
        ei = tki[:, kk]
        g = tkg[:, kk]
        for ex in range(NUM_EXPERTS):
            sel = np.where(ei == ex)[0]
            if len(sel) == 0:
                continue
            z = x[sel] @ W1[ex] + b1[ex]
            h = 0.5 * z * (1.0 + erf(z / np.sqrt(2.0)))
            out[sel] += g[sel, None] * (h @ W2[ex] + b2[ex])
    mu = out.mean(axis=1, keepdims=True)
    var = ((out - mu) ** 2).mean(axis=1, keepdims=True)
    o = (out - mu) / np.sqrt(var + LN_EPS)
    return (o * np.asarray(ln_gamma, np.float32)
            + np.asarray(ln_beta, np.float32)).astype(np.float32)


def kernel(x, edge_gate_type, edge_index, gate_type_embed, Wr, br,
           W1, b1, W2, b2, ln_gamma, ln_beta):
    b1a = np.asarray(b1); b2a = np.asarray(b2)
    ga = np.asarray(ln_gamma); ba = np.asarray(ln_beta)
    if np.any(b1a) or np.any(b2a) or np.any(ba) or not np.allclose(ga, 1.0):
        return _fallback_numpy(x, edge_gate_type, edge_index, gate_type_embed,
                               Wr, br, W1, b1, W2, b2, ln_gamma, ln_beta)

    x = np.ascontiguousarray(np.asarray(x, dtype=np.float32))
    C = _histogram(edge_index, edge_gate_type)
    eA, eB, wA, wB = _route(x, C, gate_type_embed, Wr, br)
    strips, S, S_raw, per_core = _plan(eA, eB)
    if S > MAX_S:
        return _fallback_numpy(x, edge_gate_type, edge_index, gate_type_embed,
                               Wr, br, W1, b1, W2, b2, ln_gamma, ln_beta)

    from concourse.bass_utils import run_bass_kernel_spmd

    key = ("v4", strips, S)
    if key not in _PROGRAM_CACHE:
        _PROGRAM_CACHE[key] = _build_v4(strips, S, S_raw)
    nc = _PROGRAM_CACHE[key]
    in_maps = _prep(x, eA, eB, wA, wB, W1, W2, strips, S, per_core)
    res = run_bass_kernel_spmd(nc, in_maps, core_ids=list(range(N_CORES)))

    out = np.empty((N, H), dtype=np.float32)
    for i in range(N_CORES):
        o = np.asarray(res.results[i]["outd"], dtype=np.float32)
        y_slots = o.transpose(1, 0, 2).reshape(S, H)
        slot_node, valid, _ = per_core[i]
        out[slot_node[valid]] = y_slots[valid]
    return out


# revision 7
# speedup vs baseline: 2.9936x; 2.9936x over previous
"""GateTypeExpertLayer kernel for 8 Trainium2 NeuronCores (SPMD).

v4: instruction-count-minimized design. Through this execution stack every
device instruction costs ~25-70us (measured: DVE ~26us, matmul ~69us,
DMA ~52us), so the kernel is built to minimize instruction count:

  - Host computes routing exactly (histogram -> router logits -> top-2 ->
    softmax gates) and sorts nodes by their unordered expert *pair* so each
    contiguous slot-run needs exactly 2 experts. Host also un-permutes the
    output. (The previous baseline already hosted the histogram + routing
    plan; this moves the rest of the routing bookkeeping there too.)
  - Device: per strip (<=512 slots, one expert pair): 4 W1 matmuls ->
    one batched Gelu -> one batched scale by per-slot gate weights
    (broadcast via a step-0-partition DMA load) -> 4 W2 matmuls that
    accumulate BOTH experts into one PSUM tile (the top-2 combine is free,
    done by PSUM accumulation on pre-scaled activations) -> one copy into
    a resident [128, S] bf16 accumulator.
  - One dma_start_transpose converts feat-major [128, S] to node-partition
    [128, S/128, 128]; LayerNorm runs as ~8 whole-tensor instructions; one
    contiguous DMA writes the output.

Per core: ~450 instructions total (vs ~2900 in the previous version).
"""

import numpy as np
import sys

sys.path.insert(0, "/opt/trn_rl_repo")

N_CORES = 8
N = 100000
H = 128
NUM_EXPERTS = 8
NUM_GATE_TYPES = 20
LN_EPS = 1e-5
NSH = N // N_CORES            # 12500 nodes per core
P = 128
STRIP = 512                   # max matmul free dim / PSUM bank
MAX_S = 18944                 # SBUF budget cap on padded slots per core

_PROGRAM_CACHE = {}


def _histogram(edge_index, edge_gate_type):
    dst = np.asarray(edge_index)[1].astype(np.int64)
    egt = np.asarray(edge_gate_type).astype(np.int64)
    return np.bincount(dst * NUM_GATE_TYPES + egt,
                       minlength=N * NUM_GATE_TYPES).reshape(
                           N, NUM_GATE_TYPES).astype(np.float32)


def _route(x, C, gate_type_embed, Wr, br):
    """Replicate the reference router in fp32 on host.

    Returns eA, eB (top-2 expert ids) and wA, wB (softmax gates)."""
    x = np.asarray(x, dtype=np.float32)
    G = np.asarray(gate_type_embed, dtype=np.float32)
    cnt = C.sum(axis=1, dtype=np.float32)
    gate = np.where(cnt[:, None] > 0,
                    (C @ G) / np.maximum(cnt, 1.0)[:, None],
                    0.0).astype(np.float32)
    logits = (x @ np.asarray(Wr, np.float32)
              + np.asarray(br, np.float32)[None, :] + gate)
    order = np.argsort(-logits, axis=1, kind="stable")
    eA = order[:, 0]
    eB = order[:, 1]
    lA = np.take_along_axis(logits, eA[:, None], 1)[:, 0].astype(np.float64)
    lB = np.take_along_axis(logits, eB[:, None], 1)[:, 0].astype(np.float64)
    wA = (1.0 / (1.0 + np.exp(lB - lA))).astype(np.float32)
    wB = (1.0 - wA).astype(np.float32)
    return eA, eB, wA, wB


def _plan(eA, eB):
    """Pair-sort plan shared by all cores (SPMD: one program).

    Nodes are assigned to cores round-robin *within each expert pair* (the
    host un-permutes afterwards, so any node->core assignment is legal).
    That balances pair counts exactly, so the shared per-pair capacity is
    ceil(total/8) with no alignment padding.

    Returns (strips, S, per_core): strips is a tuple of
    (offset, n, expert_a, expert_b) compile-time constants; per_core[i]
    holds (slot_node, valid, is_A_first) indexing arrays."""
    u = np.minimum(eA, eB)
    v = np.maximum(eA, eB)
    key = (u * NUM_EXPERTS + v).astype(np.int64)

    totals = np.bincount(key, minlength=NUM_EXPERTS * NUM_EXPERTS)
    cap = -(-totals // N_CORES)                      # ceil(total/8)
    cap = -(-cap // P) * P                           # 128-align each segment
    active = np.where(cap > 0)[0]

    strips = []
    seg_base = {}
    off = 0
    for kk in active:
        seg_base[int(kk)] = off
        a, b = int(kk) // NUM_EXPERTS, int(kk) % NUM_EXPERTS
        rem = int(cap[kk])
        o = off
        while rem > 0:
            n = min(STRIP, rem)
            strips.append((o, n, a, b))
            o += n
            rem -= n
        off += int(cap[kk])
    S_raw = off
    S = -(-S_raw // P) * P                           # final 128-pad only

    per_core = [[np.zeros(S, np.int64), np.zeros(S, bool), np.zeros(S, bool)]
                for _ in range(N_CORES)]
    for kk in active:
        nodes = np.where(key == kk)[0]
        base = seg_base[int(kk)]
        a = int(kk) // NUM_EXPERTS
        for i in range(N_CORES):
            sel = nodes[i::N_CORES]
            c = len(sel)
            slot_node, valid, is_A_first = per_core[i]
            slot_node[base:base + c] = sel
            valid[base:base + c] = True
            is_A_first[base:base + c] = (eA[sel] == a)
    per_core = [tuple(pc) for pc in per_core]
    return tuple(strips), S, S_raw, per_core


def _build_v4(strips, S, S_raw=None, reps=1):
    if S_raw is None:
        S_raw = S
    import concourse.bacc as bacc
    import concourse.tile as tile
    import concourse.mybir as mybir
    import concourse.bass as bass

    f32 = mybir.dt.float32
    bf16 = mybir.dt.bfloat16
    AF = mybir.ActivationFunctionType
    OP = mybir.AluOpType
    G = S // P

    nc = bacc.Bacc("TRN2", target_bir_lowering=False, debug=False,
                   num_devices=N_CORES)

    xg = nc.dram_tensor("xg", [P, S], bf16, kind="ExternalInput").ap()
    wgd = nc.dram_tensor("wgd", [2, S], bf16, kind="ExternalInput").ap()
    w1s = nc.dram_tensor("w1s", [P, 2048], bf16, kind="ExternalInput").ap()
    w2s = nc.dram_tensor("w2s", [P, 2048], bf16, kind="ExternalInput").ap()
    outd = nc.dram_tensor("outd", [P, G, H], bf16, kind="ExternalOutput").ap()

    def pbc(sl, count):
        # DRAM partition-broadcast: read one row into all partitions
        ap = [list(d) for d in sl.ap]
        return bass.AP(tensor=sl.tensor, offset=sl.offset,
                       ap=[[0, count]] + ap[1:])

    def bc(sl, count):
        ap = [list(d) for d in sl.ap]
        return bass.AP(tensor=sl.tensor, offset=sl.offset,
                       ap=ap + [[0, count]])

    def wexp(sl):
        # [P, 2, n] -> [P, 2, 2, n] with a step-0 dim for the hidden halves
        ap = [list(d) for d in sl.ap]
        return bass.AP(tensor=sl.tensor, offset=sl.offset,
                       ap=[ap[0], ap[1], [0, 2], ap[2]])

    with tile.TileContext(nc) as tc:
        with tc.tile_pool(name="const", bufs=1) as constp:
            w1_sb = constp.tile([P, 2048], bf16)
            nc.sync.dma_start(out=w1_sb[:], in_=w1s[:])
            w2_sb = constp.tile([P, 2048], bf16)
            nc.sync.dma_start(out=w2_sb[:], in_=w2s[:])
            eps_sb = constp.tile([P, 1], f32)
            nc.vector.memset(eps_sb[:], LN_EPS)
            xg_sb = constp.tile([P, S], bf16)
            nc.sync.dma_start(out=xg_sb[:], in_=xg[:])
            # per-slot gate weights broadcast to all 128 partitions
            wball = constp.tile([P, 2, S], bf16)
            nc.sync.dma_start(out=wball[:, 0, :], in_=pbc(wgd[0:1, :], P))
            nc.sync.dma_start(out=wball[:, 1, :], in_=pbc(wgd[1:2, :], P))

            def _body():
                with tc.tile_pool(name="work", bufs=1) as wp, \
                     tc.tile_pool(name="hsp", bufs=2) as hsp, \
                     tc.tile_pool(name="hpsum", bufs=1, space="PSUM") as hps, \
                     tc.tile_pool(name="ypsum", bufs=2, space="PSUM") as yps:
                    yAll = wp.tile([P, S], bf16, tag="big")
                    if S > S_raw:
                        nc.vector.memset(yAll[:, S_raw:S], 0.0)
                    for (off, n, a, b) in strips:
                        hp = hps.tile([P, 2, 2, STRIP], f32, tag="hp")
                        for ei, e in enumerate((a, b)):
                            for m in range(2):
                                nc.tensor.matmul(
                                    out=hp[:, ei, m, 0:n],
                                    lhsT=w1_sb[:, e * 256 + m * P:
                                               e * 256 + (m + 1) * P],
                                    rhs=xg_sb[:, off:off + n],
                                    start=True, stop=True)
                        hs = hsp.tile([P, 2, 2, STRIP], bf16, tag="hs")
                        nc.scalar.activation(out=hs[:, :, :, 0:n],
                                             in_=hp[:, :, :, 0:n],
                                             func=AF.Gelu)
                        nc.vector.tensor_tensor(
                            out=hs[:, :, :, 0:n], in0=hs[:, :, :, 0:n],
                            in1=wexp(wball[:, :, off:off + n]), op=OP.mult)
                        yT = yps.tile([P, STRIP], f32, tag="yT")
                        k = 0
                        for ei, e in enumerate((a, b)):
                            for m in range(2):
                                nc.tensor.matmul(
                                    out=yT[:, 0:n],
                                    lhsT=w2_sb[:, (2 * e + m) * P:
                                               (2 * e + m + 1) * P],
                                    rhs=hs[:, ei, m, 0:n],
                                    start=(k == 0), stop=(k == 3))
                                k += 1
                        nc.vector.tensor_copy(out=yAll[:, off:off + n],
                                              in_=yT[:, 0:n])

                    # ---- LayerNorm over all nodes, then store ----
                    yn = wp.tile([P, G, H], bf16, tag="yn")
                    nc.sync.dma_start_transpose(yn[:], yAll[:])
                    mu = wp.tile([P, G], f32, tag="mu")
                    nc.vector.tensor_reduce(out=mu[:], in_=yn[:],
                                            axis=mybir.AxisListType.X,
                                            op=OP.add)
                    nc.vector.tensor_scalar_mul(mu[:], mu[:], 1.0 / H)
                    nc.vector.tensor_tensor(out=yn[:], in0=yn[:],
                                            in1=bc(mu[:], H), op=OP.subtract)
                    sq = wp.tile([P, G, H], bf16, tag="big")
                    nc.scalar.activation(out=sq[:], in_=yn[:], func=AF.Square)
                    vr = wp.tile([P, G], f32, tag="vr")
                    nc.vector.tensor_reduce(out=vr[:], in_=sq[:],
                                            axis=mybir.AxisListType.X,
                                            op=OP.add)
                    sd = wp.tile([P, G], f32, tag="sd")
                    nc.scalar.activation(out=sd[:], in_=vr[:], func=AF.Sqrt,
                                         bias=eps_sb[:], scale=1.0 / H)
                    nc.vector.reciprocal(sd[:], sd[:])
                    nc.vector.tensor_tensor(out=yn[:], in0=yn[:],
                                            in1=bc(sd[:], H), op=OP.mult)
                    nc.sync.dma_start(out=outd[:], in_=yn[:])

            for _rep in range(reps):
                _body()

    nc.compile()
    return nc


def _prep(x, eA, eB, wA, wB, W1, W2, strips, S, per_core):
    import ml_dtypes
    bf = ml_dtypes.bfloat16
    x = np.asarray(x, dtype=np.float32)
    W1 = np.asarray(W1, dtype=np.float32)
    W2 = np.asarray(W2, dtype=np.float32)

    w1s = W1.transpose(1, 0, 2).reshape(P, NUM_EXPERTS * 256).astype(bf)
    w2s = W2.reshape(NUM_EXPERTS, 2, P, H).transpose(2, 0, 1, 3).reshape(
        P, NUM_EXPERTS * 256).astype(bf)

    in_maps = []
    for i in range(N_CORES):
        slot_node, valid, is_A_first = per_core[i]
        xg = np.zeros((P, S), dtype=bf)
        xg[:, valid] = x[slot_node[valid]].T.astype(bf)
        # row 0: weight of pair-min expert; row 1: weight of pair-max expert
        wgd = np.zeros((2, S), dtype=np.float32)
        wa = wA[slot_node[valid]]
        wb = wB[slot_node[valid]]
        first = is_A_first[valid]
        wgd[0, valid] = np.where(first, wa, wb)
        wgd[1, valid] = np.where(first, wb, wa)
        in_maps.append({
            "xg": np.ascontiguousarray(xg),
            "wgd": np.ascontiguousarray(wgd.astype(bf)),
            "w1s": np.ascontiguousarray(w1s),
            "w2s": np.ascontiguousarray(w2s),
        })
    return in_maps


def _fallback_numpy(x, edge_gate_type, edge_index, gate_type_embed, Wr, br,
                    W1, b1, W2, b2, ln_gamma, ln_beta):
    # exact reference recomputation on host (only for unexpected inputs)
    import math
    x = np.asarray(x, dtype=np.float32)
    n = x.shape[0]
    C = _histogram(edge_index, edge_gate_type)
    G = np.asarray(gate_type_embed, dtype=np.float32)
    cnt = C.sum(axis=1, dtype=np.float32)
    gate = np.where(cnt[:, None] > 0,
                    (C @ G) / np.maximum(cnt, 1.0)[:, None], 0.0)
    rl = x @ np.asarray(Wr, np.float32) + np.asarray(br, np.float32) + gate
    order = np.argsort(-rl, axis=1, kind="stable")
    tki = order[:, :2]
    tkl = np.take_along_axis(rl, tki, 1)
    m = tkl.max(axis=1, keepdims=True)
    e = np.exp(tkl - m)
    tkg = e / e.sum(axis=1, keepdims=True)
    W1 = np.asarray(W1, np.float32)
    b1 = np.asarray(b1, np.float32)
    W2 = np.asarray(W2, np.float32)
    b2 = np.asarray(b2, np.float32)
    out = np.zeros((n, H), np.float32)
    from scipy.special import erf  # noqa: F401  (fallback only)
    for kk in range(2):
        ei = tki[:, kk]
        g = tkg[:, kk]
        for ex in range(NUM_EXPERTS):
            sel = np.where(ei == ex)[0]
            if len(sel) == 0:
                continue
            z = x[sel] @ W1[ex] + b1[ex]
            h = 0.5 * z * (1.0 + erf(z / np.sqrt(2.0)))
            out[sel] += g[sel, None] * (h @ W2[ex] + b2[ex])
    mu = out.mean(axis=1, keepdims=True)
    var = ((out - mu) ** 2).mean(axis=1, keepdims=True)
    o = (out - mu) / np.sqrt(var + LN_EPS)
    return (o * np.asarray(ln_gamma, np.float32)
            + np.asarray(ln_beta, np.float32)).astype(np.float32)


def kernel(x, edge_gate_type, edge_index, gate_type_embed, Wr, br,
           W1, b1, W2, b2, ln_gamma, ln_beta):
    b1a = np.asarray(b1); b2a = np.asarray(b2)
    ga = np.asarray(ln_gamma); ba = np.asarray(ln_beta)
    if np.any(b1a) or np.any(b2a) or np.any(ba) or not np.allclose(ga, 1.0):
        return _fallback_numpy(x, edge_gate_type, edge_index, gate_type_embed,
                               Wr, br, W1, b1, W2, b2, ln_gamma, ln_beta)

    x = np.ascontiguousarray(np.asarray(x, dtype=np.float32))
    C = _histogram(edge_index, edge_gate_type)
    eA, eB, wA, wB = _route(x, C, gate_type_embed, Wr, br)
    strips, S, S_raw, per_core = _plan(eA, eB)
    if S > MAX_S:
        return _fallback_numpy(x, edge_gate_type, edge_index, gate_type_embed,
                               Wr, br, W1, b1, W2, b2, ln_gamma, ln_beta)

    from concourse.bass_utils import run_bass_kernel_spmd

    key = ("v4", strips, S)
    if key not in _PROGRAM_CACHE:
        _PROGRAM_CACHE[key] = _build_v4(strips, S, S_raw)
    nc = _PROGRAM_CACHE[key]
    in_maps = _prep(x, eA, eB, wA, wB, W1, W2, strips, S, per_core)
    res = run_bass_kernel_spmd(nc, in_maps, core_ids=list(range(N_CORES)))

    out = np.empty((N, H), dtype=np.float32)
    for i in range(N_CORES):
        o = np.asarray(res.results[i]["outd"], dtype=np.float32)
        y_slots = o.transpose(1, 0, 2).reshape(S, H)
        slot_node, valid, _ = per_core[i]
        out[slot_node[valid]] = y_slots[valid]
    return out
